# revision 20
# baseline (speedup 1.0000x reference)
"""AdditiveAttention Trainium2 kernel.

reference:
    wx = vector @ w_matrix                        [B, D]
    uy = matrix @ u_matrix                        [B, S, D]
    scores = tanh(wx[:, None, :] + uy) @ v        [B, S]
    out = softmax(scores, axis=-1)

B, S, D = 32, 2048, 1024.  Data-parallel over B across 8 NeuronCores
(4 batches/core), params replicated.  Per core everything is computed in
the transposed layout z^T[d', s] (d' on partitions) so that:
  - the +wx and tanh fuse into one ScalarE activation (per-partition bias)
  - the v-dot is a TensorE matmul contracting over partitions, emitting
    scores[b, s] directly (batch b lands on partition b via a padded-v
    stationary operand), so softmax is a cheap free-axis op.
matrix must be transposed on-chip (contraction dim d must sit on
partitions): done with regular PE matmuls against an identity, in bf16.
The f32->bf16 cast of matrix happens inside the load DMA (SWDGE cast).
"""

import ml_dtypes
import numpy as np

import concourse.bass as bass
import concourse.mybir as mybir
import concourse.tile as tile
from concourse import bacc
from concourse.bass_utils import run_bass_kernel_spmd
from concourse.masks import make_identity

N_CORES = 8
B, S, D = 32, 2048, 1024
BB = B // N_CORES  # batches per core = 4
F32 = mybir.dt.float32
BF16 = mybir.dt.bfloat16
NST = S // 512  # s-tiles of 512 per batch = 4
NSQ = 4         # 128-row subtiles per s-tile
NKC = D // 128  # contraction chunks = 8
NDT = D // 128  # d' tiles = 8


def _body(nc, tc, x_d, y_d, w_d, u_d, v_d, o_d):
    from contextlib import ExitStack

    with ExitStack() as ctx:
        const = ctx.enter_context(tc.tile_pool(name="const", bufs=1))
        yb_pool = ctx.enter_context(tc.tile_pool(name="yb", bufs=3))
        yt_pool = ctx.enter_context(tc.tile_pool(name="yt", bufs=20))
        inter_pool = ctx.enter_context(tc.tile_pool(name="inter", bufs=24))
        ps_t = ctx.enter_context(tc.tile_pool(name="ps_t", bufs=2, space="PSUM"))
        ps_z = ctx.enter_context(tc.tile_pool(name="ps_z", bufs=2, space="PSUM"))
        ps_sc = ctx.enter_context(tc.tile_pool(name="ps_sc", bufs=2, space="PSUM"))
        

        ident = const.tile([128, 128], BF16)
        make_identity(nc, ident[:])

        # ---- params arrive already bf16 (host-cast); HWDGE queue, no casts.
        # x and v first (tiny, unblock the wx chain and vp tiles), then U
        # (earliest big PE consumer), then W.
        xb = const.tile([BB, D], BF16)
        nc.sync.dma_start(xb[:], x_d[:, :])
        v_bf = const.tile([128, NKC], BF16)
        nc.sync.dma_start(
            v_bf[:], v_d[:, :].rearrange("(c p) o -> p (c o)", p=128))
        u_bf = []
        for c in range(NKC):
            ub = const.tile([128, D], BF16, tag=f"u{c}")
            nc.sync.dma_start(ub[:], u_d[bass.ds(128 * c, 128), :])
            u_bf.append(ub)
        w_bf = []
        for c in range(NKC):
            wb = const.tile([128, D], BF16, tag=f"w{c}")
            nc.sync.dma_start(wb[:], w_d[bass.ds(128 * c, 128), :])
            w_bf.append(wb)

        # per-batch padded stationary tiles vp[b]: [128, 32*NDT],
        # block t col b = v chunk t, rest 0.  Built on ScalarE (idle at
        # startup) so the DVE queue stays free for yt evacuations.
        vp = []
        for b in range(BB):
            t_vp = const.tile([128, 32 * NDT], BF16, tag=f"vp{b}")
            nc.vector.memset(t_vp[:], 0.0)
            for t in range(NDT):
                nc.scalar.copy(t_vp[:, 32 * t + b : 32 * t + b + 1], v_bf[:, t : t + 1])
            vp.append(t_vp)

        # ---- wx^T[d', b] ----
        xT = const.tile([128, 4 * NKC], BF16)  # chunk c at cols 4c..4c+4
        for c in range(NKC):
            p_x = ps_sc.tile([128, 4], F32, tag="ps_sc")
            nc.tensor.matmul(p_x[:], xb[:, bass.ds(128 * c, 128)], ident[:BB, :BB],
                             start=True, stop=True)
            nc.vector.tensor_copy(xT[:, 4 * c : 4 * c + 4], p_x[:])
        wxT = const.tile([128, 4 * NDT], F32)  # d'-tile t at cols 4t..4t+4
        for t in range(NDT):
            p_wx = ps_sc.tile([128, 4], F32, tag="ps_sc")
            for c in range(NKC):
                nc.tensor.matmul(p_wx[:], w_bf[c][:, bass.ds(128 * t, 128)],
                                 xT[:, bass.ds(4 * c, 4)],
                                 start=(c == 0), stop=(c == NKC - 1))
            nc.vector.tensor_copy(wxT[:, 4 * t : 4 * t + 4], p_wx[:])

        probs = const.tile([BB, S], F32)
        s_all = const.tile([BB, NST], F32)

        # ---- main loops ----
        # v-dots for batch b are emitted as one contiguous block one batch
        # late (between batch b+1's PE work), so the M=32<->M=128 PE
        # reconfiguration penalty is paid twice per batch instead of 16x,
        # and the PE never waits on ScalarE's tanh output.
        def emit_vdots(p_sc, b, inters, last):
            for t in range(NDT):
                nc.tensor.matmul(p_sc[:], vp[b][:, bass.ds(32 * t, 32)],
                                 inters[t][:],
                                 start=(b == 0 and t == 0),
                                 stop=(last and t == NDT - 1))

        pending_copy = None
        for st in range(NST):
            p_sc = ps_sc.tile([32, 512], F32, tag="ps_sc")
            pending = []
            for bp in range(BB // 2):
                b0, b1 = 2 * bp, 2 * bp + 1
                ybs, ytss = [], []
                for b in (b0, b1):
                    row0 = b * S + st * 512
                    yb = yb_pool.tile([128, NSQ * D], BF16, tag="yb")
                    nc.gpsimd.dma_start(
                        yb[:].rearrange("p (q d) -> p q d", q=NSQ),
                        y_d[bass.ds(row0, 512), :].rearrange("(q p) d -> p q d", p=128),
                    )
                    yts = []
                    for c in range(NKC):
                        p_t = ps_t.tile([128, 512], BF16, tag="ps_t")
                        for q in range(NSQ):
                            nc.tensor.matmul(p_t[:, bass.ds(128 * q, 128)],
                                             yb[:, bass.ds(D * q + 128 * c, 128)], ident[:],
                                             start=(q == 0), stop=(q == NSQ - 1),
                                             is_transpose=True)
                        yt = yt_pool.tile([128, 512], BF16, tag="yt")
                        nc.vector.tensor_copy(yt[:], p_t[:])
                        yts.append(yt)
                    ytss.append(yts)
                inters_pair = ([], [])
                for t in range(NDT):
                    pz0 = ps_z.tile([128, 512], F32, tag="ps_z")
                    pz1 = ps_z.tile([128, 512], F32, tag="ps_z1")
                    for c in range(NKC):
                        w_ap = u_bf[c][:, bass.ds(128 * t, 128)]
                        nc.tensor.matmul(pz0[:], w_ap, ytss[0][c][:],
                                         start=(c == 0), stop=(c == NKC - 1))
                        nc.tensor.matmul(pz1[:], w_ap, ytss[1][c][:],
                                         start=(c == 0), stop=(c == NKC - 1))
                    for i, (b, p_z) in enumerate(((b0, pz0), (b1, pz1))):
                        inter = inter_pool.tile([128, 512], BF16, tag="inter")
                        nc.scalar.activation(inter[:], p_z[:],
                                             mybir.ActivationFunctionType.Tanh,
                                             bias=wxT[:, 4 * t + b : 4 * t + b + 1],
                                             scale=1.0)
                        inters_pair[i].append(inter)
                    if t == 0 and bp == 0 and pending_copy is not None:
                        ps, pp = pending_copy
                        nc.scalar.activation(
                            probs[:, bass.ds(512 * ps, 512)], pp[:BB, :],
                            mybir.ActivationFunctionType.Exp,
                            accum_out=s_all[:, ps : ps + 1])
                        pending_copy = None
                    if t == 0 and bp == 1:
                        for pb, pi in pending:
                            emit_vdots(p_sc, pb, pi, last=False)
                        pending = []
                pending.append((b0, inters_pair[0]))
                pending.append((b1, inters_pair[1]))
            for i, (pb, pi) in enumerate(pending):
                emit_vdots(p_sc, pb, pi, last=(i == len(pending) - 1))
            pending_copy = (st, p_sc)
        ps, pp = pending_copy
        nc.scalar.activation(probs[:, bass.ds(512 * ps, 512)], pp[:BB, :],
                             mybir.ActivationFunctionType.Exp,
                             accum_out=s_all[:, ps : ps + 1])

        # ---- normalize: scores are bounded (|s| <= ||v||_1 ~ 39) so the
        # max-subtraction is unnecessary in f32; row sums were folded into
        # the per-tile Exp via accum_out.
        tot = const.tile([BB, 1], F32)
        nc.vector.tensor_reduce(tot[:], s_all[:], axis=mybir.AxisListType.X,
                                op=mybir.AluOpType.add)
        rinv = const.tile([BB, 1], F32)
        nc.vector.reciprocal(rinv[:], tot[:])
        out_sb = const.tile([BB, S], F32)
        nc.vector.tensor_scalar_mul(out_sb[:], probs[:], rinv[:, :])
        nc.sync.dma_start(o_d[:, :], out_sb[:])


_CACHED_NC = None


def _get_nc():
    global _CACHED_NC
    if _CACHED_NC is None:
        nc = bacc.Bacc("TRN2", target_bir_lowering=False, debug=False,
                       enable_asserts=False, num_devices=N_CORES)
        x_d = nc.dram_tensor("vector", [BB, D], BF16, kind="ExternalInput").ap()
        y_d = nc.dram_tensor("matrix", [BB * S, D], F32, kind="ExternalInput").ap()
        w_d = nc.dram_tensor("w_matrix", [D, D], BF16, kind="ExternalInput").ap()
        u_d = nc.dram_tensor("u_matrix", [D, D], BF16, kind="ExternalInput").ap()
        v_d = nc.dram_tensor("v_vector", [D, 1], BF16, kind="ExternalInput").ap()
        o_d = nc.dram_tensor("out", [BB, S], F32, kind="ExternalOutput").ap()
        with tile.TileContext(nc) as tc:
            _body(nc, tc, x_d, y_d, w_d, u_d, v_d, o_d)
        nc.compile()
        _CACHED_NC = nc
    return _CACHED_NC


def kernel(vector, matrix, w_matrix, u_matrix, v_vector, _run_kwargs=None):
    nc = _get_nc()
    bf = ml_dtypes.bfloat16
    vector = np.ascontiguousarray(np.asarray(vector, dtype=np.float32).astype(bf))
    matrix = np.ascontiguousarray(np.asarray(matrix, dtype=np.float32))
    w_matrix = np.ascontiguousarray(np.asarray(w_matrix, dtype=np.float32).astype(bf))
    u_matrix = np.ascontiguousarray(np.asarray(u_matrix, dtype=np.float32).astype(bf))
    v_vector = np.ascontiguousarray(np.asarray(v_vector, dtype=np.float32).astype(bf))
    in_maps = []
    for i in range(N_CORES):
        b0 = i * BB
        in_maps.append({
            "vector": vector[b0 : b0 + BB],
            "matrix": np.ascontiguousarray(matrix[b0 : b0 + BB].reshape(BB * S, D)),
            "w_matrix": w_matrix,
            "u_matrix": u_matrix,
            "v_vector": v_vector,
        })
    res = run_bass_kernel_spmd(nc, in_maps, core_ids=list(range(N_CORES)),
                               **(_run_kwargs or {}))
    out = np.concatenate([res.results[i]["out"] for i in range(N_CORES)], axis=0)
    if _run_kwargs is not None:
        kernel.last_result = res
    return out


# revision 21
# speedup vs baseline: 1.1854x; 1.1854x over previous
"""AdditiveAttention Trainium2 kernel.

reference:
    wx = vector @ w_matrix                        [B, D]
    uy = matrix @ u_matrix                        [B, S, D]
    scores = tanh(wx[:, None, :] + uy) @ v        [B, S]
    out = softmax(scores, axis=-1)

B, S, D = 32, 2048, 1024.  Data-parallel over B across 8 NeuronCores
(4 batches/core), params replicated.  Per core everything is computed in
the transposed layout z^T[d', s] (d' on partitions) so that:
  - the +wx and tanh fuse into one ScalarE activation (per-partition bias)
  - the v-dot is a TensorE matmul contracting over partitions, emitting
    scores[b, s] directly (batch b lands on partition b via a padded-v
    stationary operand), so softmax is a cheap free-axis op.
matrix must be transposed on-chip (contraction dim d must sit on
partitions): done with regular PE matmuls against an identity, in bf16.
The f32->bf16 cast of matrix happens inside the load DMA (SWDGE cast).
"""

import ml_dtypes
import numpy as np

import concourse.bass as bass
import concourse.mybir as mybir
import concourse.tile as tile
from concourse import bacc
from concourse.bass_utils import run_bass_kernel_spmd
from concourse.masks import make_identity

N_CORES = 8
B, S, D = 32, 2048, 1024
BB = B // N_CORES  # batches per core = 4
F32 = mybir.dt.float32
BF16 = mybir.dt.bfloat16
NST = S // 512  # s-tiles of 512 per batch = 4
NSQ = 4         # 128-row subtiles per s-tile
NKC = D // 128  # contraction chunks = 8
NDT = D // 128  # d' tiles = 8


def _body(nc, tc, x_d, y_d, w_d, u_d, v_d, o_d):
    from contextlib import ExitStack

    with ExitStack() as ctx:
        const = ctx.enter_context(tc.tile_pool(name="const", bufs=1))
        yb_pool = ctx.enter_context(tc.tile_pool(name="yb", bufs=3))
        yt_pool = ctx.enter_context(tc.tile_pool(name="yt", bufs=20))
        inter_pool = ctx.enter_context(tc.tile_pool(name="inter", bufs=24))
        ps_t = ctx.enter_context(tc.tile_pool(name="ps_t", bufs=2, space="PSUM"))
        ps_z = ctx.enter_context(tc.tile_pool(name="ps_z", bufs=2, space="PSUM"))
        ps_sc = ctx.enter_context(tc.tile_pool(name="ps_sc", bufs=2, space="PSUM"))
        

        # U per-chunk tiles via SWDGE, emitted first on the Q7 ring so they
        # land ahead of the y stream (the first z-group is gated on U).
        u_bf = []
        for c in range(NKC):
            ub = const.tile([128, D], BF16, tag=f"u{c}")
            nc.gpsimd.dma_start(ub[:], u_d[bass.ds(128 * c, 128), :])
            u_bf.append(ub)

        ident = const.tile([128, 128], BF16)
        make_identity(nc, ident[:])

        # ---- params arrive already bf16 (host-cast); HWDGE queue, no casts.
        # x and v first (tiny, unblock the wx chain and vp tiles), then U
        # (earliest big PE consumer), then W.
        xb = const.tile([BB, D], BF16)
        nc.sync.dma_start(xb[:], x_d[:, :])
        v_bf = const.tile([128, NKC], BF16)
        nc.sync.dma_start(
            v_bf[:], v_d[:, :].rearrange("(c p) o -> p (c o)", p=128))
        w_bf = []
        for c in range(NKC):
            wb = const.tile([128, D], BF16, tag=f"w{c}")
            nc.sync.dma_start(wb[:], w_d[bass.ds(128 * c, 128), :])
            w_bf.append(wb)

        # per-batch padded stationary tiles vp[b]: [128, 32*NDT],
        # block t col b = v chunk t, rest 0.  Built on ScalarE (idle at
        # startup) so the DVE queue stays free for yt evacuations.
        vp = []
        for b in range(BB):
            t_vp = const.tile([128, 32 * NDT], BF16, tag=f"vp{b}")
            nc.vector.memset(t_vp[:], 0.0)
            for t in range(NDT):
                nc.scalar.copy(t_vp[:, 32 * t + b : 32 * t + b + 1], v_bf[:, t : t + 1])
            vp.append(t_vp)

        # ---- wx^T[d', b] ----
        xT = const.tile([128, 4 * NKC], BF16)  # chunk c at cols 4c..4c+4
        for c in range(NKC):
            p_x = ps_sc.tile([128, 4], F32, tag="ps_sc")
            nc.tensor.matmul(p_x[:], xb[:, bass.ds(128 * c, 128)], ident[:BB, :BB],
                             start=True, stop=True)
            nc.vector.tensor_copy(xT[:, 4 * c : 4 * c + 4], p_x[:])
        wxT = const.tile([128, 4 * NDT], F32)  # d'-tile t at cols 4t..4t+4
        for t in range(NDT):
            p_wx = ps_sc.tile([128, 4], F32, tag="ps_sc")
            for c in range(NKC):
                nc.tensor.matmul(p_wx[:], w_bf[c][:, bass.ds(128 * t, 128)],
                                 xT[:, bass.ds(4 * c, 4)],
                                 start=(c == 0), stop=(c == NKC - 1))
            nc.vector.tensor_copy(wxT[:, 4 * t : 4 * t + 4], p_wx[:])

        probs = const.tile([BB, S], F32)
        s_all = const.tile([BB, NST], F32)

        # ---- main loops ----
        # v-dots for batch b are emitted as one contiguous block one batch
        # late (between batch b+1's PE work), so the M=32<->M=128 PE
        # reconfiguration penalty is paid twice per batch instead of 16x,
        # and the PE never waits on ScalarE's tanh output.
        def emit_vdots(p_sc, b, inters, last):
            for t in range(NDT):
                nc.tensor.matmul(p_sc[:], vp[b][:, bass.ds(32 * t, 32)],
                                 inters[t][:],
                                 start=(b == 0 and t == 0),
                                 stop=(last and t == NDT - 1))

        pending_copy = None
        for st in range(NST):
            p_sc = ps_sc.tile([32, 512], F32, tag="ps_sc")
            pending = []
            for bp in range(BB // 2):
                b0, b1 = 2 * bp, 2 * bp + 1
                ybs, ytss = [], []
                for b in (b0, b1):
                    row0 = b * S + st * 512
                    yb = yb_pool.tile([128, NSQ * D], BF16, tag="yb")
                    nc.gpsimd.dma_start(
                        yb[:].rearrange("p (q d) -> p q d", q=NSQ),
                        y_d[bass.ds(row0, 512), :].rearrange("(q p) d -> p q d", p=128),
                    )
                    yts = []
                    for c in range(NKC):
                        p_t = ps_t.tile([128, 512], BF16, tag="ps_t")
                        for q in range(NSQ):
                            nc.tensor.matmul(p_t[:, bass.ds(128 * q, 128)],
                                             yb[:, bass.ds(D * q + 128 * c, 128)], ident[:],
                                             start=(q == 0), stop=(q == NSQ - 1),
                                             is_transpose=True)
                        yt = yt_pool.tile([128, 512], BF16, tag="yt")
                        nc.vector.tensor_copy(yt[:], p_t[:])
                        yts.append(yt)
                    ytss.append(yts)
                inters_pair = ([], [])
                for t in range(NDT):
                    pz0 = ps_z.tile([128, 512], F32, tag="ps_z")
                    pz1 = ps_z.tile([128, 512], F32, tag="ps_z1")
                    for c in range(NKC):
                        w_ap = u_bf[c][:, bass.ds(128 * t, 128)]
                        nc.tensor.matmul(pz0[:], w_ap, ytss[0][c][:],
                                         start=(c == 0), stop=(c == NKC - 1))
                        nc.tensor.matmul(pz1[:], w_ap, ytss[1][c][:],
                                         start=(c == 0), stop=(c == NKC - 1))
                    for i, (b, p_z) in enumerate(((b0, pz0), (b1, pz1))):
                        inter = inter_pool.tile([128, 512], BF16, tag="inter")
                        nc.scalar.activation(inter[:], p_z[:],
                                             mybir.ActivationFunctionType.Tanh,
                                             bias=wxT[:, 4 * t + b : 4 * t + b + 1],
                                             scale=1.0)
                        inters_pair[i].append(inter)
                    if t == 0 and bp == 0 and pending_copy is not None:
                        ps, pp = pending_copy
                        nc.scalar.activation(
                            probs[:, bass.ds(512 * ps, 512)], pp[:BB, :],
                            mybir.ActivationFunctionType.Exp,
                            accum_out=s_all[:, ps : ps + 1])
                        pending_copy = None
                    if t == 0 and bp == 1:
                        for pb, pi in pending:
                            emit_vdots(p_sc, pb, pi, last=False)
                        pending = []
                pending.append((b0, inters_pair[0]))
                pending.append((b1, inters_pair[1]))
            for i, (pb, pi) in enumerate(pending):
                emit_vdots(p_sc, pb, pi, last=(i == len(pending) - 1))
            pending_copy = (st, p_sc)
        ps, pp = pending_copy
        nc.scalar.activation(probs[:, bass.ds(512 * ps, 512)], pp[:BB, :],
                             mybir.ActivationFunctionType.Exp,
                             accum_out=s_all[:, ps : ps + 1])

        # ---- normalize: scores are bounded (|s| <= ||v||_1 ~ 39) so the
        # max-subtraction is unnecessary in f32; row sums were folded into
        # the per-tile Exp via accum_out.
        tot = const.tile([BB, 1], F32)
        nc.vector.tensor_reduce(tot[:], s_all[:], axis=mybir.AxisListType.X,
                                op=mybir.AluOpType.add)
        rinv = const.tile([BB, 1], F32)
        nc.vector.reciprocal(rinv[:], tot[:])
        out_sb = const.tile([BB, S], F32)
        nc.vector.tensor_scalar_mul(out_sb[:], probs[:], rinv[:, :])
        nc.sync.dma_start(o_d[:, :], out_sb[:])


_CACHED_NC = None


def _get_nc():
    global _CACHED_NC
    if _CACHED_NC is None:
        nc = bacc.Bacc("TRN2", target_bir_lowering=False, debug=False,
                       enable_asserts=False, num_devices=N_CORES)
        x_d = nc.dram_tensor("vector", [BB, D], BF16, kind="ExternalInput").ap()
        y_d = nc.dram_tensor("matrix", [BB * S, D], F32, kind="ExternalInput").ap()
        w_d = nc.dram_tensor("w_matrix", [D, D], BF16, kind="ExternalInput").ap()
        u_d = nc.dram_tensor("u_matrix", [D, D], BF16, kind="ExternalInput").ap()
        v_d = nc.dram_tensor("v_vector", [D, 1], BF16, kind="ExternalInput").ap()
        o_d = nc.dram_tensor("out", [BB, S], F32, kind="ExternalOutput").ap()
        with tile.TileContext(nc) as tc:
            _body(nc, tc, x_d, y_d, w_d, u_d, v_d, o_d)
        nc.compile()
        _CACHED_NC = nc
    return _CACHED_NC


def kernel(vector, matrix, w_matrix, u_matrix, v_vector, _run_kwargs=None):
    nc = _get_nc()
    bf = ml_dtypes.bfloat16
    vector = np.ascontiguousarray(np.asarray(vector, dtype=np.float32).astype(bf))
    matrix = np.ascontiguousarray(np.asarray(matrix, dtype=np.float32))
    w_matrix = np.ascontiguousarray(np.asarray(w_matrix, dtype=np.float32).astype(bf))
    u_matrix = np.ascontiguousarray(np.asarray(u_matrix, dtype=np.float32).astype(bf))
    v_vector = np.ascontiguousarray(np.asarray(v_vector, dtype=np.float32).astype(bf))
    in_maps = []
    for i in range(N_CORES):
        b0 = i * BB
        in_maps.append({
            "vector": vector[b0 : b0 + BB],
            "matrix": np.ascontiguousarray(matrix[b0 : b0 + BB].reshape(BB * S, D)),
            "w_matrix": w_matrix,
            "u_matrix": u_matrix,
            "v_vector": v_vector,
        })
    res = run_bass_kernel_spmd(nc, in_maps, core_ids=list(range(N_CORES)),
                               **(_run_kwargs or {}))
    out = np.concatenate([res.results[i]["out"] for i in range(N_CORES)], axis=0)
    if _run_kwargs is not None:
        kernel.last_result = res
    return out
